# revision 1
# baseline (speedup 1.0000x reference)
"""Trainium2 Bass kernel for MultiHeadAttention with relative-position bias.

Problem shapes: N=4, S=1024, H=1024, NH=16, D=64, P=20 (clamp window).
Returns (out, ctx) like the reference.

Sharding: 8 cores; core c handles batch n=c//2, head-group hg=c%2 (8 heads).
Each core computes its heads' QKV projections, attention, the ctx column
slice, and a partial out (row-sharded Wo contraction). Host sums the two
partials per batch and adds bo.

Device-side structure:
  - Inputs arrive host-transposed (h-major) so projections contract over h
    directly; matmuls run in float32r (1 cycle/row at moving>=256); the
    attention-weight/V path runs in bf16 (random rounding averages out).
  - energy_pos[q,k] = Q[q]*rel_emb[clip(q-k,-20,20)+20]: B = Q @ rel_emb^T
    rides in the same PSUM tile as QK^T; the far-field column enters the
    fused exp as a per-partition bias; the 20-wide near-diagonal correction
    is placed by a diagonal-access-pattern DMA onto a causal-mask template.
  - Softmax without max-subtraction (energies are O(0.3)); the row sum is
    fused into the exp (accum_out); normalization is deferred to the
    per-partition-scaled ctx eviction in phase C.
  - P is transposed on the PE in q-block pairs for wide AV matmuls; ctx^T
    is re-transposed once more after normalization to feed the output
    projection with a 128-deep contraction.
"""

import sys

if "/opt/trn_rl_repo" not in sys.path:
    sys.path.insert(0, "/opt/trn_rl_repo")

import numpy as np

import concourse.bass as bass
import concourse.mybir as mybir
import concourse.tile as tile
from concourse import bacc
from concourse.bass_utils import run_bass_kernel_spmd

F32 = mybir.dt.float32
F32R = mybir.dt.float32r
AF = mybir.ActivationFunctionType

S = 1024
D = 64
NHG = 8      # heads per core
NPAIR = 4    # head pairs per core
HC = 8       # 128-row contraction chunks over H
SB = 8       # 128-row blocks over S
BCOL = 982   # column where the 42 B-columns live inside the S psum tile
MASKV = -1.0e9
WIN = 147    # band window width (19 + 128)


def _chunks(w):
    out = [(0, min(w, 512))]
    if w > 512:
        out.append((512, w))
    return out


def build_nc():
    nc = bacc.Bacc("TRN2", target_bir_lowering=False, debug=False)

    BF16 = mybir.dt.bfloat16
    xqT = nc.dram_tensor("xqT", (S, S), BF16, kind="ExternalInput").ap()
    xkT = nc.dram_tensor("xkT", (S, S), BF16, kind="ExternalInput").ap()
    xvT = nc.dram_tensor("xvT", (S, S), BF16, kind="ExternalInput").ap()
    wq = nc.dram_tensor("wq", (S, 512), BF16, kind="ExternalInput").ap()
    wk = nc.dram_tensor("wk", (S, 512), BF16, kind="ExternalInput").ap()
    wv = nc.dram_tensor("wv", (S, 512), BF16, kind="ExternalInput").ap()
    wo = nc.dram_tensor("wo", (512, S), BF16, kind="ExternalInput").ap()
    bq2 = nc.dram_tensor("bq2", (128, 4), F32, kind="ExternalInput").ap()
    bk2 = nc.dram_tensor("bk2", (128, 4), F32, kind="ExternalInput").ap()
    bvr = nc.dram_tensor("bvr", (1, 512), F32R, kind="ExternalInput").ap()
    relTr = nc.dram_tensor("relTr", (128, 42), BF16, kind="ExternalInput").ap()

    o_part = nc.dram_tensor("o_part", (S, S), BF16,
                            kind="ExternalOutput").ap()
    ctx_out = nc.dram_tensor("ctx_out", (S, 512), BF16,
                             kind="ExternalOutput").ap()

    import ml_dtypes
    ident_np = np.eye(128, dtype=np.float32)
    templ_np = np.zeros((128, WIN), dtype=np.float32)
    for p in range(128):
        templ_np[p, p + 20:] = MASKV
    templ_np = templ_np.astype(ml_dtypes.bfloat16)
    ident_d = nc.inline_tensor(ident_np, name="ident_c")
    identb_d = nc.inline_tensor(ident_np.astype(ml_dtypes.bfloat16),
                                name="identb_c")
    templ_d = nc.inline_tensor(templ_np, name="templ_c")
    ones_d = nc.inline_tensor(np.ones((1, 128), np.float32), name="ones_c")
    import ml_dtypes as _mld
    sel_np = np.zeros((2, 16, 512), np.float32)
    for _tt in range(2):
        for _h in range(8):
            sel_np[_tt, 2 * _h + _tt, _h * 64:(_h + 1) * 64] = 1.0
    sel0_d = nc.inline_tensor(sel_np[0].astype(_mld.bfloat16),
                              name="sel0_c")
    sel1_d = nc.inline_tensor(sel_np[1].astype(_mld.bfloat16),
                              name="sel1_c")
    zeros_d = nc.inline_tensor(np.zeros((128, 128), np.float32),
                               name="zeros_c")


    # greedy ACT/DVE balance for PSUM->SBUF evictions.
    # Pre-loaded with the fixed per-engine work (ACT: exp ~56us;
    # DVE: band adds/src/Ball/recip ~30us) so copies land fairly.
    ebusy = {"act": 115000.0, "dve": 0.0}

    def _pick(cact, cdve):
        if ebusy["act"] + cact < ebusy["dve"] + cdve:
            ebusy["act"] += cact
            return "act"
        ebusy["dve"] += cdve
        return "dve"

    def ecopy(out, in_, cols, bf=False):
        if _pick(cols * 0.833 + 450.0,
                 cols * (0.521 if bf else 1.042) + 295.0) == "act":
            nc.scalar.copy(out, in_)
        else:
            nc.vector.tensor_copy(out, in_)

    def escale(out, in_, scale, cols):
        if _pick(cols * 0.833 + 450.0, cols * 1.042 + 295.0) == "act":
            nc.scalar.activation(out, in_, AF.Copy, scale=scale)
        else:
            nc.vector.tensor_scalar_mul(out, in_, scale)

    def ebias(out, in_, bias, cols):
        if _pick(cols * 0.833 + 450.0, cols * 1.042 + 295.0) == "act":
            nc.scalar.activation(out, in_, AF.Identity, bias=bias)
        else:
            nc.vector.tensor_scalar_add(out, in_, bias)

    with tile.TileContext(nc) as tc:
        import contextlib

        with contextlib.ExitStack() as ctx:
            ep = ctx.enter_context
            cpool = ep(tc.tile_pool(name="consts", bufs=1))
            templ = cpool.tile([128, WIN], BF16, tag="templ")
            nc.sync.dma_start(templ[:], templ_d.ap())
            relT = cpool.tile([128, 42], BF16, tag="relT")
            nc.sync.dma_start(relT[:], relTr)
            bq_sb = cpool.tile([128, 4], F32, tag="bq")
            nc.sync.dma_start(bq_sb[:], bq2)
            bk_sb = cpool.tile([128, 4], F32, tag="bk")
            nc.sync.dma_start(bk_sb[:], bk2)
            ident = cpool.tile([128, 128], F32R, tag="ident")
            bv_sb = cpool.tile([1, 512], F32R, tag="bv")
            ones = cpool.tile([1, 128], F32R, tag="ones")
            zero128 = cpool.tile([128, 128], BF16, tag="zero128")
            identb = cpool.tile([128, 128], BF16, tag="identb")
            sel0 = cpool.tile([16, 512], BF16, tag="sel0")
            sel1 = cpool.tile([16, 512], BF16, tag="sel1")

            def load_late_consts():
                nc.sync.dma_start(identb[:], identb_d.ap())
                nc.sync.dma_start(zero128[:],
                                  zeros_d.ap().bitcast(BF16)[:, 0:128])
                nc.sync.dma_start(ones[:], ones_d.ap().bitcast(F32R))
                nc.sync.dma_start(bv_sb[:], bvr)
                nc.sync.dma_start(ident[:], ident_d.ap().bitcast(F32R))
                nc.sync.dma_start(sel0[:], sel0_d.ap())
                nc.sync.dma_start(sel1[:], sel1_d.ap())

            big = ep(tc.tile_pool(name="big", bufs=1))
            qT = big.tile([128, NPAIR, S], BF16, tag="qT", name="qT")[:]
            kT = big.tile([128, NPAIR, S], BF16, tag="kT", name="kT")[:]
            vN = big.tile([128, SB, 512], BF16, tag="vN", name="vN")[:]
            stg_h = []
            for _i in range(NHG):
                _t = big.tile([128, SB, WIN], BF16, tag=f"stg{_i}",
                              name=f"stg{_i}")
                stg_h.append(_t[:])

            # PSUM pools: 2*2 (S) + 3 (general) + 1 (AV) = 8 banks
            spp = ep(tc.tile_pool(name="spp", bufs=2, space="PSUM"))
            gpp = ep(tc.tile_pool(name="gpp", bufs=3, space="PSUM"))
            cxp = ep(tc.tile_pool(name="cxp", bufs=1, space="PSUM"))

            # SBUF working pools (coexist with xT/wx below)
            pbuf = ep(tc.tile_pool(name="pbuf", bufs=14))
            ptbuf = ep(tc.tile_pool(name="ptbuf", bufs=4))
            cujp = ep(tc.tile_pool(name="cujp", bufs=2))
            cns = ep(tc.tile_pool(name="cns", bufs=3))
            ctp = ep(tc.tile_pool(name="ctp", bufs=3))
            osb = ep(tc.tile_pool(name="osb", bufs=3))
            small = ep(tc.tile_pool(name="small", bufs=6))
            rjbp = ep(tc.tile_pool(name="rjbp", bufs=2))
            bsm = ep(tc.tile_pool(name="bsm", bufs=12))
            xTp = ep(tc.tile_pool(name="xTp", bufs=1))
            wxp = ep(tc.tile_pool(name="wxp", bufs=2))

            # ---------------- Phase A: loads + projections + pre-pass -------
            def load_input(xdram, wdram):
                w_sb = wxp.tile([128, HC, 512], BF16, tag="wx", name="w_sb")[:]
                wre = wdram.rearrange("(c p) n -> p c n", p=128)
                xT = xTp.tile([128, HC, S], BF16, tag="xT", name="xT")[:]
                for hc in range(HC):
                    nc.sync.dma_start(w_sb[:, hc, :], wre[:, hc, :])
                    nc.sync.dma_start(xT[:, hc, :],
                                      xdram[hc * 128:(hc + 1) * 128, :])
                return xT, w_sb

            def proj_qk(xT, w_sb, outT, b_sb):
                for pair in range(NPAIR):
                    for qc in range(2):
                        pp = gpp.tile([128, 512], F32, tag="gp", name="pp")
                        for hc in range(HC):
                            nc.tensor.matmul(
                                pp[:],
                                w_sb[:, hc, pair * 128:(pair + 1) * 128],
                                xT[:, hc, qc * 512:(qc + 1) * 512],
                                start=(hc == 0), stop=(hc == HC - 1))
                        ebias(outT[:, pair, qc * 512:(qc + 1) * 512],
                              pp[:], b_sb[:, pair:pair + 1], 512)

            # Q first (pre-pass depends on it); K rides in the P-pool
            # slots (same shape, idle until attention) so its load is not
            # serialized behind the xT slot.
            xTq, w_q = load_input(xqT, wq)
            xkc = []
            for hc in range(HC):
                xk1 = pbuf.tile([128, 1024], BF16, tag="P", name=f"xk{hc}")
                nc.sync.dma_start(xk1[:], xkT[hc * 128:(hc + 1) * 128, :])
                xkc.append(xk1[:])
            w_k = wxp.tile([128, HC, 512], BF16, tag="wx", name="w_k")[:]
            wkre = wk.rearrange("(c p) n -> p c n", p=128)
            for hc in range(HC):
                nc.sync.dma_start(w_k[:, hc, :], wkre[:, hc, :])

            proj_qk(xTq, w_q, qT, bq_sb)

            # fill all staging tiles with the causal-mask template up front
            for h in range(NHG):
                for t in range(SB):
                    nc.gpsimd.tensor_copy(stg_h[h][:, t, :], templ[:])

            # band pre-pass: B = Q @ rel^T, staging tiles + biases
            def prepass(ts_):
                for t in ts_:
                    for h in range(NHG):
                        pairb, halfb = divmod(h, 2)
                        idx = h * SB + t
                        bp = gpp.tile([128, 512], F32, tag="gp", name="bp")
                        nc.tensor.matmul(
                            bp[:, 0:42],
                            qT[64 * halfb:64 * halfb + 64, pairb,
                               t * 128:(t + 1) * 128],
                            relT[64 * halfb:64 * halfb + 64, :],
                            start=True, stop=True)
                        srcb = bsm.tile([128, 20], BF16, tag="srcb")
                        nc.vector.tensor_scalar(
                            srcb[:], bp[:, 1:21], bp[:, 0:1], 8.0,
                            mybir.AluOpType.subtract,
                            mybir.AluOpType.mult)
                        stga = stg_h[h][:, t, :]
                        diag = bass.AP(
                            stga.tensor, stga.offset,
                            [[SB * WIN + 1, 128], [1, 20]])
                        if idx % 2 == 0:
                            nc.sync.dma_start(diag, srcb[:])
                        else:
                            nc.gpsimd.dma_start(diag, srcb[:])

            # K projection from the P-slot chunks
            for pair in range(NPAIR):
                for qc in range(2):
                    pp = gpp.tile([128, 512], F32, tag="gp", name="pp")
                    for hc in range(HC):
                        nc.tensor.matmul(
                            pp[:],
                            w_k[:, hc, pair * 128:(pair + 1) * 128],
                            xkc[hc][:, qc * 512:(qc + 1) * 512],
                            start=(hc == 0), stop=(hc == HC - 1))
                    ebias(kT[:, pair, qc * 512:(qc + 1) * 512],
                          pp[:], bk_sb[:, pair:pair + 1], 512)

            prepass((0, 1))

            xTv, w_v = load_input(xvT, wv)
            load_late_consts()

            def v_chain(kb):
                pp = gpp.tile([128, 512], F32, tag="gp", name="pp")
                for hc in range(HC):
                    nc.tensor.matmul(
                        pp[:],
                        xTv[:, hc, kb * 128:(kb + 1) * 128],
                        w_v[:, hc, :],
                        start=(hc == 0), stop=False)
                nc.tensor.matmul(pp[:], ones[:], bv_sb[:],
                                 start=False, stop=True)
                ecopy(vN[:, kb, :], pp[:], 512)

            prepass((2, 3))

            # ------ attention per q-pair j, staged batches with lookahead ---
            state = {}

            def s_batch(j, hb):
                rj, cuj, Pt = state[j]
                for h in (hb, hb + 1):
                    pair, half = divmod(h, 2)
                    qTh = qT[64 * half:64 * half + 64]
                    kTh = kT[64 * half:64 * half + 64]
                    for tt in (0, 1):
                        t = 2 * j + tt
                        W = 128 * (t + 1)
                        sp = spp.tile([128, 1024], F32, tag="sp", name="sp")
                        lhs = qTh[:, pair, t * 128:(t + 1) * 128]
                        w0 = max(0, 128 * t - 19)
                        stga = stg_h[h][:, t, :]
                        # plain region [0, w0), split at the 512 psum bank
                        c = 0
                        while c < w0:
                            c1 = min(w0, 512) if c < 512 else w0
                            nc.tensor.matmul(sp[:, c:c1], lhs,
                                             kTh[:, pair, c:c1],
                                             start=True, stop=True)
                            c = c1
                        # band region [w0, W): scores + identity-matmul add
                        # of the stg band (causal mask + rel-pos correction)
                        pieces = ([(w0, W)] if (w0 >= 512 or W <= 512)
                                  else [(w0, 512), (512, W)])
                        for (a, b) in pieces:
                            nc.tensor.matmul(sp[:, a:b], lhs,
                                             kTh[:, pair, a:b],
                                             start=True, stop=False)
                            s0 = a - (128 * t - 19)
                            nc.tensor.matmul(sp[:, a:b], identb[:],
                                             stga[:, s0:s0 + (b - a)],
                                             start=False, stop=True)
                        P = pbuf.tile([128, 1024], BF16, tag="P", name="P")
                        sums = small.tile([128, 1], F32, tag="sums")
                        nc.scalar.activation(
                            P[:, 0:W], sp[:, 0:W], AF.Exp,
                            scale=1.0 / 64.0, accum_out=sums[:])
                        with nc.allow_low_precision(
                                reason="f32r out is f32 bits"):
                            nc.vector.reciprocal(
                                rj[:, h, tt:tt + 1], sums[:])
                        Pt[(h, tt)] = P

            def t_batch(j, hb):
                rj, cuj, Pt = state[j]
                for h in (hb, hb + 1):
                    P0 = Pt.pop((h, 0))
                    P1 = Pt.pop((h, 1))
                    pT = ptbuf.tile([128, 2 * j + 2, 256], BF16,
                                    tag="pT", name="pT")[:]
                    Pt[("pT", h)] = pT
                    for kb2 in range(0, 2 * j + 2, 2):
                        pt = gpp.tile([128, 512], BF16, tag="gp", name="pt")
                        for i in range(2):
                            kb = kb2 + i
                            # kb == 2j+1: that P0 quarter is never read
                            # by the AV matmuls - skip its transpose
                            if kb <= 2 * j:
                                nc.tensor.transpose(
                                    pt[:, i * 256:i * 256 + 128],
                                    P0[:, kb * 128:(kb + 1) * 128],
                                    identb[:])
                            nc.tensor.transpose(
                                pt[:, i * 256 + 128:i * 256 + 256],
                                P1[:, kb * 128:(kb + 1) * 128],
                                identb[:])
                        ecopy(pT[:, kb2:kb2 + 2, :], pt[:, 0:512], 512, bf=True)

            def av_batch(j, hb):
                rj, cuj, Pt = state[j]
                for h in (hb, hb + 1):
                    pT = Pt.pop(("pT", h))
                    cx = cxp.tile([64, 256], F32, tag="cx")
                    for kb in range(2 * j + 1):
                        nc.tensor.matmul(
                            cx[:], vN[:, kb, h * 64:(h + 1) * 64],
                            pT[:, kb, :],
                            start=(kb == 0), stop=False)
                    nc.tensor.matmul(
                        cx[:, 128:256],
                        vN[:, 2 * j + 1, h * 64:(h + 1) * 64],
                        pT[:, 2 * j + 1, 128:256],
                        start=False, stop=True)
                    ecopy(cuj[:, h, 0:256], cx[:], 256)

            def materialize_rjb(j):
                rj, cuj, Pt = state[j]
                rtp = gpp.tile([128, 512], F32R, tag="gp", name="rtp")
                rjf = bass.AP(rj.tensor, rj.offset,
                              [[NHG * 2, 128], [1, NHG * 2]])
                nc.tensor.transpose(rtp[0:16, 0:128], rjf, ident[:])
                rjT = small.tile([16, 128], BF16, tag="rjT", name="rjT")
                ecopy(rjT[:], rtp[0:16, 0:128], 128)
                rjb = rjbp.tile([128, 2, 512], BF16, tag="rjb",
                                name="rjb")[:]
                Pt["rjb"] = rjb
                for tt in (0, 1):
                    rb = gpp.tile([128, 512], F32, tag="gp", name="rb")
                    nc.tensor.matmul(
                        rb[:], rjT[:], (sel0 if tt == 0 else sel1)[:],
                        start=True, stop=True)
                    ecopy(rjb[:, tt, :], rb[:], 512, bf=True)

            def output_stage(j):
                rj, cuj, Pt = state.pop(j)
                rjb = Pt.pop("rjb")
                for tt in (0, 1):
                    qb = 2 * j + tt
                    cnall = gpp.tile([128, 512], BF16, tag="gp", name="cnall")
                    for h in range(NHG):
                        nc.tensor.transpose(
                            cnall[:, h * 64:(h + 1) * 64],
                            cuj[:, h, tt * 128:(tt + 1) * 128],
                            identb[0:64, 0:64])
                    cn = cns.tile([128, 512], BF16, tag="cn")
                    nc.vector.tensor_tensor(cn[:], cnall[:],
                                            rjb[:, tt, :],
                                            mybir.AluOpType.mult)
                    nc.sync.dma_start(
                        ctx_out[qb * 128:(qb + 1) * 128, :], cn[:])
                    rt = gpp.tile([128, 512], BF16, tag="gp", name="rt")
                    for pc in range(NPAIR):
                        nc.tensor.transpose(
                            rt[:, pc * 128:(pc + 1) * 128],
                            cn[:, pc * 128:(pc + 1) * 128],
                            identb[:])
                    ctxT = ctp.tile([128, NPAIR, 128], BF16, tag="ctxT")
                    ecopy(ctxT[:], rt[:], 512, bf=True)
                    ou = osb.tile([128, 1024], BF16, tag="ou")
                    for oc in range(2):
                        op = spp.tile([128, 1024], F32, tag="sp", name="op")
                        for pc in range(NPAIR):
                            nc.tensor.matmul(
                                op[:, 0:512],
                                ctxT[:, pc, :],
                                wo_sb[:, pc, oc * 512:(oc + 1) * 512],
                                start=(pc == 0), stop=(pc == NPAIR - 1))
                        ecopy(ou[:, oc * 512:(oc + 1) * 512],
                              op[:, 0:512], 512)
                    nc.sync.dma_start(o_part[qb * 128:(qb + 1) * 128, :],
                                      ou[:])

            def new_state(j):
                rj = cujp.tile([128, NHG, 2], F32R, tag="rj", name="rj")[:]
                cuj = cujp.tile([64, NHG, 256], BF16, tag="cuj",
                                name="cuj")[:]
                state[j] = (rj, cuj, {})

            # flat schedule: adjacent q-pairs interleaved, V chains and
            # the deferred wo load woven in, output stages lagging.
            new_state(0)
            s_batch(0, 0)
            s_batch(0, 2)
            v_chain(0)
            t_batch(0, 0)
            s_batch(0, 4)
            v_chain(1)
            t_batch(0, 2)
            v_chain(2)
            av_batch(0, 0)
            s_batch(0, 6)
            v_chain(3)
            t_batch(0, 4)
            av_batch(0, 2)
            v_chain(4)
            prepass((4, 5))
            new_state(1)
            s_batch(1, 0)
            v_chain(5)
            t_batch(0, 6)
            materialize_rjb(0)
            av_batch(0, 4)
            v_chain(6)
            s_batch(1, 2)
            v_chain(7)
            av_batch(0, 6)
            # wo reuses the (now free) xT slot
            wo_sb = xTp.tile([128, NPAIR, S], BF16, tag="xT",
                             name="wo_sb")[:]
            nc.sync.dma_start(wo_sb,
                              wo.rearrange("(c p) n -> p c n", p=128))
            t_batch(1, 0)
            s_batch(1, 4)
            t_batch(1, 2)
            av_batch(1, 0)
            s_batch(1, 6)
            t_batch(1, 4)
            av_batch(1, 2)
            output_stage(0)
            prepass((6, 7))
            new_state(2)
            s_batch(2, 0)
            t_batch(1, 6)
            materialize_rjb(1)
            av_batch(1, 4)
            s_batch(2, 2)
            av_batch(1, 6)
            t_batch(2, 0)
            s_batch(2, 4)
            t_batch(2, 2)
            av_batch(2, 0)
            output_stage(1)
            s_batch(2, 6)
            new_state(3)
            t_batch(2, 4)
            av_batch(2, 2)
            s_batch(3, 0)
            t_batch(2, 6)
            materialize_rjb(2)
            av_batch(2, 4)
            s_batch(3, 2)
            av_batch(2, 6)
            t_batch(3, 0)
            s_batch(3, 4)
            t_batch(3, 2)
            av_batch(3, 0)
            output_stage(2)
            s_batch(3, 6)
            t_batch(3, 4)
            av_batch(3, 2)
            t_batch(3, 6)
            materialize_rjb(3)
            av_batch(3, 4)
            av_batch(3, 6)
            output_stage(3)

    nc.compile()
    return nc


_NC = None


def _get_nc():
    global _NC
    if _NC is None:
        _NC = build_nc()
    return _NC


def make_in_maps(query, key, value, Wq, bq, Wk, bk, Wv, bv, Wo, rel_emb):
    import ml_dtypes
    BF = ml_dtypes.bfloat16
    asf = lambda a: np.ascontiguousarray(a, dtype=np.float32)
    asb = lambda a: np.ascontiguousarray(np.asarray(a, np.float32),
                                         dtype=BF)
    r1 = asf(rel_emb.T[:, ::-1])
    r1 = np.concatenate([r1, np.zeros((64, 1), np.float32)], axis=1)
    relTr = np.ascontiguousarray(
        np.concatenate([r1, r1], axis=0).astype(BF))
    # far-field rel-pos bias folded into bk: energy far field is
    # Q.(K/64 + e40/8) = Q.(K + 8*e40)/64 with e40 = rel_emb[2P].
    bk_f = np.asarray(bk, np.float32) + np.tile(
        np.asarray(rel_emb[-1], np.float32) * 8.0, 16)
    in_maps = []
    for c in range(8):
        n, hg = divmod(c, 2)
        cs = slice(512 * hg, 512 * (hg + 1))
        in_maps.append({
            "xqT": asb(np.asarray(query[n]).T),
            "xkT": asb(np.asarray(key[n]).T),
            "xvT": asb(np.asarray(value[n]).T),
            "wq": asb(Wq[:, cs]),
            "wk": asb(Wk[:, cs]),
            "wv": asb(Wv[:, cs]),
            "wo": asb(Wo[cs, :]),
            "bq2": asf(np.asarray(bq)[cs].reshape(4, 128).T),
            "bk2": asf(bk_f[cs].reshape(4, 128).T),
            "bvr": asf(np.asarray(bv)[cs].reshape(1, 512)),
            "relTr": relTr,
        })
    return in_maps


def run(inputs, trace=False, trace_kwargs=None):
    nc = _get_nc()
    in_maps = make_in_maps(
        np.asarray(inputs["query"]), np.asarray(inputs["key"]),
        np.asarray(inputs["value"]), np.asarray(inputs["Wq"]),
        np.asarray(inputs["bq"]), np.asarray(inputs["Wk"]),
        np.asarray(inputs["bk"]), np.asarray(inputs["Wv"]),
        np.asarray(inputs["bv"]), np.asarray(inputs["Wo"]),
        np.asarray(inputs["rel_emb"]))
    kw = {}
    if trace:
        kw["trace"] = True
        if trace_kwargs:
            kw.update(trace_kwargs)
    res = run_bass_kernel_spmd(nc, in_maps, core_ids=list(range(8)), **kw)
    bo = np.asarray(inputs["bo"], dtype=np.float32)
    out = np.zeros((4, S, S), np.float32)
    ctx = np.zeros((4, S, S), np.float32)
    for c in range(8):
        n, hg = divmod(c, 2)
        out[n] += res.results[c]["o_part"]
        ctx[n][:, 512 * hg:512 * (hg + 1)] = res.results[c]["ctx_out"]
    out += bo
    return (out, ctx), res


def kernel(**inputs):
    (out, ctx), _ = run(inputs)
    return (out, ctx)

